# revision 6
# baseline (speedup 1.0000x reference)
"""CGConv layer (gather -> GEMM -> scatter-mean) on 8 Trainium2 NeuronCores.

Strategy: destination-node sharding. Nodes are assigned to 8 cores x 112
windows of 56 nodes each (degree-balanced). Each core processes exactly the
edges that point at its windows, aggregates [x_j | edge_attr | 1] per window
with one-hot matmuls on the PE (bf16 operands, f32 PSUM accumulation), then
applies the 192x128 weight at node granularity in f32, divides by counts and
adds bias. The x-gather runs on the GPSIMD SWDGE dma_gather path (int16
indices -> x split into lo/hi tables) in 1024-index chunks over 4 queues.
"""
import sys

sys.path.insert(0, "/opt/trn_rl_repo")

import contextlib
import ctypes
import types
from contextlib import ExitStack

import numpy as np
import ml_dtypes

N_CORES = 8
N_NODES = 50000
OUT_CH = 128
EDGE_CH = 64
WIN = 56                     # dest nodes per window
W_PER_CORE = 112             # windows per core
N_WIN = N_CORES * W_PER_CORE
SPLIT = 32768                # x row split for int16 gather indices
CHUNK = 1024                 # dma_gather indices per instruction (ring limit)
ND = W_PER_CORE * WIN        # dest slots per core (6272)

_prog_cache = {}
last_exec_time_ns = None


def _install_ntff_hook():
    """Optional: lets BASS_TRACE=1 produce exec_time_ns under axon."""
    if "antenv.axon_hooks" in sys.modules:
        return
    try:
        lib = ctypes.CDLL("/opt/axon/libaxon_pjrt.so")
        if not hasattr(lib, "axon_start_nrt_profile"):
            return
        lib.axon_start_nrt_profile.argtypes = [
            ctypes.POINTER(ctypes.c_int64), ctypes.c_size_t]
        lib.axon_start_nrt_profile.restype = ctypes.c_int64
        lib.axon_stop_nrt_profile.argtypes = [ctypes.c_char_p]
        lib.axon_stop_nrt_profile.restype = ctypes.c_int64

        @contextlib.contextmanager
        def _hook(output_dir, device_ids):
            import jax
            jax.devices()
            if device_ids:
                ids = (ctypes.c_int64 * len(device_ids))(*device_ids)
                rc = lib.axon_start_nrt_profile(ids, len(device_ids))
            else:
                rc = lib.axon_start_nrt_profile(None, 0)
            if rc != 0:
                raise RuntimeError(f"axon_start_nrt_profile rc={rc}")
            try:
                yield
            finally:
                n = lib.axon_stop_nrt_profile(str(output_dir).encode())
                print(f"profile: {n} file(s) in {output_dir}", file=sys.stderr)

        mod = types.ModuleType("antenv.axon_hooks")
        mod.get_axon_ntff_profile_hook = lambda: _hook
        mod.set_axon_ntff_profile_hook = lambda h: None
        sys.modules["antenv.axon_hooks"] = mod
    except Exception:
        pass


def _build_program(locap, hicap):
    from concourse import bacc, tile
    import concourse.mybir as mybir
    from concourse.library_config import mlp
    import concourse.tile_sem_assignment as _tsa

    # Pin each Pool-engine DMA's DMASW lane to its SWDGE queue so the
    # completion semaphore is only ever incremented from that queue
    # (Tile's default round-robin lane assignment is queue-oblivious).
    if not getattr(_tsa, "_queue_lane_patch", False):
        _orig_assign = _tsa.TileClockTick._assign_tick

        def _assign_tick(self, inst):
            if (isinstance(inst, _tsa.DMAInst)
                    and inst.engine == mybir.EngineType.Pool):
                self.next_sw_dma_idx = getattr(inst, "queue_num", 0) or 0
            return _orig_assign(self, inst)

        _tsa.TileClockTick._assign_tick = _assign_tick
        _tsa._queue_lane_patch = True

    bf16 = mybir.dt.bfloat16
    f32 = mybir.dt.float32
    i16 = mybir.dt.int16
    eq = mybir.AluOpType.is_equal

    tlw = locap // 128           # lo tiles per window
    thw = hicap // 128           # hi tiles per window
    s_lo = W_PER_CORE * locap    # lo-region slots
    s_hi = W_PER_CORE * hicap
    s_tot = s_lo + s_hi
    nch_lo = s_lo // CHUNK
    nch_hi = s_hi // CHUNK
    t_lo = s_lo // 128           # lo-region tiles
    n_grp = W_PER_CORE // 4      # psum groups of 4 windows

    nc = bacc.Bacc("TRN2", target_bir_lowering=False, debug=False,
                   num_devices=N_CORES, num_swdge_queues=4)
    xlo_d = nc.dram_tensor("xlo", [SPLIT, OUT_CH], bf16, kind="ExternalInput")
    xhi_d = nc.dram_tensor("xhi", [N_NODES - SPLIT, OUT_CH], bf16,
                           kind="ExternalInput")
    e65_d = nc.dram_tensor("e65", [s_tot, EDGE_CH + 1], bf16,
                           kind="ExternalInput")
    idxlo_d = nc.dram_tensor("idxlo", [128, s_lo // 16], i16,
                             kind="ExternalInput")
    idxhi_d = nc.dram_tensor("idxhi", [128, s_hi // 16], i16,
                             kind="ExternalInput")
    drel_d = nc.dram_tensor("drel", [128, s_tot // 128], f32,
                            kind="ExternalInput")
    iota_d = nc.dram_tensor("iota", [128, WIN], bf16, kind="ExternalInput")
    w1_d = nc.dram_tensor("w1ext", [128, 129], f32, kind="ExternalInput")
    w2_d = nc.dram_tensor("w2ext", [65, 129], f32, kind="ExternalInput")
    bias_d = nc.dram_tensor("biasb", [128, OUT_CH], f32, kind="ExternalInput")
    outp_d = nc.dram_tensor("outp", [ND, OUT_CH], f32, kind="ExternalOutput")

    with tile.TileContext(nc) as tc, ExitStack() as ctx:
        const = ctx.enter_context(tc.tile_pool(name="const", bufs=1))
        glo = ctx.enter_context(tc.tile_pool(name="glo", bufs=4))
        ghi = ctx.enter_context(tc.tile_pool(name="ghi", bufs=4))
        epool = ctx.enter_context(tc.tile_pool(name="e65", bufs=3))
        ohpool = ctx.enter_context(tc.tile_pool(name="oh", bufs=3))
        agg = ctx.enter_context(tc.tile_pool(name="agg", bufs=1))
        pspool = ctx.enter_context(tc.tile_pool(name="ps", bufs=3, space="PSUM"))
        ps2pool = ctx.enter_context(tc.tile_pool(name="ps2", bufs=2, space="PSUM"))
        fin = ctx.enter_context(tc.tile_pool(name="fin", bufs=3))

        nc.gpsimd.load_library(mlp)

        idxlo_t = const.tile([128, s_lo // 16], i16)
        nc.sync.dma_start(idxlo_t[:], idxlo_d[:])
        idxhi_t = const.tile([128, s_hi // 16], i16)
        nc.sync.dma_start(idxhi_t[:], idxhi_d[:])
        drel_t = const.tile([128, s_tot // 128], f32)
        nc.sync.dma_start(drel_t[:], drel_d[:])
        iota_t = const.tile([128, WIN], bf16)
        nc.sync.dma_start(iota_t[:], iota_d[:])
        w1_t = const.tile([128, 129], f32)
        nc.sync.dma_start(w1_t[:], w1_d[:])
        w2_t = const.tile([65, 129], f32)
        nc.sync.dma_start(w2_t[:], w2_d[:])
        bias_t = const.tile([128, OUT_CH], f32)
        nc.sync.dma_start(bias_t[:], bias_d[:])

        aggx = agg.tile([128, ND], f32, tag="aggx")
        agge = agg.tile([65, ND], f32, tag="agge")

        lo_tiles, hi_tiles = {}, {}
        qctr = [0]

        def lo_chunk(j):
            if j not in lo_tiles:
                t = glo.tile([128, 8, OUT_CH], bf16, tag="glo")
                if j < 4:
                    nc.vector.memset(t[:], 0.0)
                nc.gpsimd.dma_gather(
                    t[:], xlo_d[:], idxlo_t[:, j * 64:(j + 1) * 64],
                    CHUNK, CHUNK, OUT_CH, queue_num=qctr[0] % 4)
                qctr[0] += 1
                lo_tiles[j] = t
            return lo_tiles[j]

        def hi_chunk(j):
            if j not in hi_tiles:
                t = ghi.tile([128, 8, OUT_CH], bf16, tag="ghi")
                if j < 4:
                    nc.vector.memset(t[:], 0.0)
                nc.gpsimd.dma_gather(
                    t[:], xhi_d[:], idxhi_t[:, j * 64:(j + 1) * 64],
                    CHUNK, CHUNK, OUT_CH, queue_num=qctr[0] % 4)
                qctr[0] += 1
                hi_tiles[j] = t
            return hi_tiles[j]

        for g in range(n_grp):
            ps_t = pspool.tile([128, 4 * 112], f32, tag="ps")

            # one-hot blocks for this group's lo and hi tiles
            oh_lo = ohpool.tile([128, 4 * tlw, WIN], bf16, tag="ohlo")
            nc.vector.tensor_tensor(
                out=oh_lo[:],
                in0=iota_t[:].unsqueeze(1).to_broadcast([128, 4 * tlw, WIN]),
                in1=drel_t[:, g * 4 * tlw:(g + 1) * 4 * tlw]
                    .unsqueeze(2).to_broadcast([128, 4 * tlw, WIN]),
                op=eq)
            oh_hi = ohpool.tile([128, 4 * thw, WIN], bf16, tag="ohhi")
            nc.vector.tensor_tensor(
                out=oh_hi[:],
                in0=iota_t[:].unsqueeze(1).to_broadcast([128, 4 * thw, WIN]),
                in1=drel_t[:, t_lo + g * 4 * thw:t_lo + (g + 1) * 4 * thw]
                    .unsqueeze(2).to_broadcast([128, 4 * thw, WIN]),
                op=eq)

            # edge features (+ ones column) for this group's slots
            elo = epool.tile([128, 4 * tlw, EDGE_CH + 1], bf16, tag="elo")
            nc.sync.dma_start(
                elo[:],
                e65_d[g * 4 * locap:(g + 1) * 4 * locap, :]
                .rearrange("(t p) f -> p t f", p=128))
            ehi = epool.tile([128, 4 * thw, EDGE_CH + 1], bf16, tag="ehi")
            nc.sync.dma_start(
                ehi[:],
                e65_d[s_lo + g * 4 * hicap:s_lo + (g + 1) * 4 * hicap, :]
                .rearrange("(t p) f -> p t f", p=128))

            for wi in range(4):
                w = 4 * g + wi
                xcol = slice(wi * 112, wi * 112 + WIN)
                ecol = slice(wi * 112 + WIN, wi * 112 + 2 * WIN)
                # x aggregation: 5 lo tiles then 3 hi tiles, one group
                for i in range(tlw):
                    tg = w * tlw + i
                    nc.tensor.matmul(ps_t[:, xcol],
                                     lo_chunk(tg // 8)[:, tg % 8, :],
                                     oh_lo[:, wi * tlw + i, :],
                                     start=(i == 0), stop=False)
                for i in range(thw):
                    tg = w * thw + i
                    nc.tensor.matmul(ps_t[:, xcol],
                                     hi_chunk(tg // 8)[:, tg % 8, :],
                                     oh_hi[:, wi * thw + i, :],
                                     start=False, stop=(i == thw - 1))
                # edge-attr (+ones) aggregation: its own group
                for i in range(tlw):
                    nc.tensor.matmul(ps_t[0:65, ecol],
                                     elo[:, wi * tlw + i, :],
                                     oh_lo[:, wi * tlw + i, :],
                                     start=(i == 0), stop=False)
                for i in range(thw):
                    nc.tensor.matmul(ps_t[0:65, ecol],
                                     ehi[:, wi * thw + i, :],
                                     oh_hi[:, wi * thw + i, :],
                                     start=False, stop=(i == thw - 1))

            # harvest psum -> f32 aggregates (strided picks of x / e halves)
            psv = ps_t[:].rearrange("p (w f) -> p w f", f=112)
            nc.scalar.mul(
                aggx[:, g * 4 * WIN:(g + 1) * 4 * WIN]
                .rearrange("p (w f) -> p w f", f=WIN),
                psv[:, :, 0:WIN], 1.0)
            nc.scalar.mul(
                agge[0:65, g * 4 * WIN:(g + 1) * 4 * WIN]
                .rearrange("p (w f) -> p w f", f=WIN),
                psv[0:65, :, WIN:2 * WIN], 1.0)

        # GEMM2 + mean + bias, 128 dest rows at a time
        import concourse.mybir as mybir
        for d in range(ND // 128):
            ps2 = ps2pool.tile([128, 129], f32, tag="ps2")
            nc.tensor.matmul(ps2[:], aggx[:, d * 128:(d + 1) * 128], w1_t[:],
                             start=True, stop=False)
            nc.tensor.matmul(ps2[:], agge[:, d * 128:(d + 1) * 128], w2_t[:],
                             start=False, stop=True)
            cnt = fin.tile([128, 1], f32, tag="cnt")
            nc.vector.tensor_scalar_max(cnt[:], ps2[:, 128:129], 1.0)
            rec = fin.tile([128, 1], f32, tag="rec")
            nc.vector.reciprocal(rec[:], cnt[:])
            stage = fin.tile([128, OUT_CH], f32, tag="stage")
            nc.vector.scalar_tensor_tensor(
                out=stage[:], in0=ps2[:, 0:128], scalar=rec[:, 0:1],
                in1=bias_t[:], op0=mybir.AluOpType.mult,
                op1=mybir.AluOpType.add)
            nc.sync.dma_start(outp_d[d * 128:(d + 1) * 128, :], stage[:])

    nc.compile()
    return nc


def kernel(x, edge_index, edge_attr, weight, bias):
    global last_exec_time_ns
    _install_ntff_hook()
    from concourse.bass_utils import run_bass_kernel_spmd

    x = np.asarray(x, dtype=np.float32)
    edge_index = np.asarray(edge_index, dtype=np.int64)
    edge_attr = np.asarray(edge_attr, dtype=np.float32)
    weight = np.asarray(weight, dtype=np.float32)
    bias = np.asarray(bias, dtype=np.float32)
    row, col = edge_index[0], edge_index[1]
    E = row.shape[0]

    # ---- window assignment: stratified by lo-degree, hi balanced in-stratum
    lo_edge = col < SPLIT
    lodeg = np.bincount(row[lo_edge], minlength=N_NODES).astype(np.int64)
    hideg = np.bincount(row[~lo_edge], minlength=N_NODES).astype(np.int64)
    order = np.argsort(-lodeg, kind="stable")
    node_window = np.empty(N_NODES, np.int64)
    node_pos = np.empty(N_NODES, np.int64)
    win_fill = np.zeros(N_WIN, np.int64)
    win_hi = np.zeros(N_WIN, np.int64)
    n_strata = (N_NODES + N_WIN - 1) // N_WIN
    for s in range(n_strata):
        stratum = order[s * N_WIN:(s + 1) * N_WIN]
        nodes_sorted = stratum[np.argsort(-hideg[stratum], kind="stable")]
        wins_sorted = np.argsort(win_hi, kind="stable")[:len(nodes_sorted)]
        node_window[nodes_sorted] = wins_sorted
        win_hi[wins_sorted] += hideg[nodes_sorted]
        node_pos[nodes_sorted] = win_fill[wins_sorted]
        win_fill[wins_sorted] += 1

    ew = node_window[row]                       # global window per edge
    edest = node_pos[row].astype(np.float64)    # local dest (0..55)

    n_lo_w = np.bincount(ew[lo_edge], minlength=N_WIN)
    n_hi_w = np.bincount(ew[~lo_edge], minlength=N_WIN)
    locap = max(128, int(-(-n_lo_w.max() // 128)) * 128)
    hicap = max(128, int(-(-n_hi_w.max() // 128)) * 128)

    s_lo = W_PER_CORE * locap
    s_hi = W_PER_CORE * hicap
    s_tot = s_lo + s_hi

    # ---- per-edge slot in its core's region-ordered layout
    def positions(mask):
        w = ew[mask]
        o = np.argsort(w, kind="stable")
        sw = w[o]
        starts = np.searchsorted(sw, np.arange(N_WIN))
        pos = np.empty(len(sw), np.int64)
        pos[o] = np.arange(len(sw)) - starts[sw]
        return pos

    slot = np.empty(E, np.int64)
    pl = positions(lo_edge)
    slot[lo_edge] = (ew[lo_edge] % W_PER_CORE) * locap + pl
    ph = positions(~lo_edge)
    slot[~lo_edge] = s_lo + (ew[~lo_edge] % W_PER_CORE) * hicap + ph
    ecore = ew // W_PER_CORE

    # ---- shared input tables
    xlo = x[:SPLIT].astype(ml_dtypes.bfloat16)
    xhi = x[SPLIT:].astype(ml_dtypes.bfloat16)
    w1ext = np.zeros((128, 129), np.float32)
    w1ext[:, :128] = weight[:128]
    w2ext = np.zeros((65, 129), np.float32)
    w2ext[:64, :128] = weight[128:]
    w2ext[64, 128] = 1.0
    biasb = np.broadcast_to(bias, (128, OUT_CH)).copy().astype(np.float32)
    iota = np.broadcast_to(np.arange(WIN, dtype=np.float32),
                           (128, WIN)).astype(ml_dtypes.bfloat16).copy()

    def wrap16(values, slots, region_size):
        a = np.full((16, region_size // 16), -1, np.int16)
        a[slots % 16, slots // 16] = values
        return np.tile(a, (8, 1))

    in_maps = []
    for c in range(N_CORES):
        m = ecore == c
        sl = slot[m]
        e65 = np.zeros((s_tot, EDGE_CH + 1), ml_dtypes.bfloat16)
        e65[sl, :EDGE_CH] = edge_attr[m].astype(ml_dtypes.bfloat16)
        e65[sl, EDGE_CH] = 1.0
        drel = np.full((128, s_tot // 128), 999.0, np.float32)
        drel[sl % 128, sl // 128] = edest[m]
        mlo = m & lo_edge
        mhi = m & ~lo_edge
        idxlo = wrap16(col[mlo].astype(np.int16), slot[mlo], s_lo)
        idxhi = wrap16((col[mhi] - SPLIT).astype(np.int16),
                       slot[mhi] - s_lo, s_hi)
        in_maps.append({
            "xlo": xlo, "xhi": xhi, "e65": e65,
            "idxlo": idxlo, "idxhi": idxhi, "drel": drel,
            "iota": iota, "w1ext": w1ext, "w2ext": w2ext, "biasb": biasb,
        })

    key = (locap, hicap)
    if key not in _prog_cache:
        _prog_cache[key] = _build_program(locap, hicap)
    nc = _prog_cache[key]

    res = run_bass_kernel_spmd(nc, in_maps, list(range(N_CORES)))
    last_exec_time_ns = res.exec_time_ns

    allres = np.stack([res.results[c]["outp"] for c in range(N_CORES)])
    out = allres[node_window // W_PER_CORE,
                 (node_window % W_PER_CORE) * WIN + node_pos]
    return np.ascontiguousarray(out.astype(np.float32))
